# revision 1
# baseline (speedup 1.0000x reference)
"""Trainium2 Bass kernel for nn_MultiHeadedAttention (B=2, S=2048, D=1024, H=16).

Sharding (8 NeuronCores): tensor-parallel over heads x data-parallel over batch.
Core c handles batch b = c // 4 and head group g = c % 4 (4 heads = 256 dims).

Per-core pipeline (all matmuls bf16, fp32 PSUM accumulation):
  - Q^T/K^T projections in transposed layout [e, s], contraction(kt)-OUTER so
    matmuls start as soon as the first x-tile DMA lands (8 PSUM banks live).
    Per-partition bias via DVE on eviction; 1/sqrt(dk) folded into Wq host-side.
  - Scores computed transposed, S^T = K @ Q^T, per (512-q-chunk, 128-k-tile)
    with causally dead leading columns trimmed off the matmul and PV reads.
    exp on ScalarE straight out of PSUM (scores bounded, no max-subtraction).
  - Mask applied as data on GPSIMD (SBUF-only engine): 0/1 multiply per
    diagonal/arbitrary block; fully-masked blocks skipped at build time.
  - V projection [s, e] augmented with 64 REPLICATED ones columns per head:
    the PV matmul then emits the denominator replicated across partitions
    64-127, so normalization is a plain [64,512] reciprocal + multiply on DVE
    (no partition-broadcast DMA round trip).
  - PV accumulates X^T per (q-chunk, head) with V_aug stationary.
  - Output projection per q-chunk interleaved behind attention; evictions
    split ScalarE/DVE; host reduces the 4 head-group partials per batch + bo.
"""

from contextlib import ExitStack

import numpy as np
import ml_dtypes

import concourse.bass as bass  # noqa: F401
import concourse.bacc as bacc
import concourse.tile as tile
import concourse.mybir as mybir
from concourse.bass_utils import run_bass_kernel_spmd

dt = mybir.dt
AFT = mybir.ActivationFunctionType
BF16 = ml_dtypes.bfloat16

B, S, D, H = 2, 2048, 1024, 16
DK = D // H                  # 64
NCORES = 8
G = 4                        # heads per core
E = G * DK                   # 256 projected dims per core
QC_W = 512                   # q chunk width
KT_W = 128                   # k tile width
NQC = S // QC_W              # 4
NKT = S // KT_W              # 16
NMS = S // 128               # 16 s-tiles
NXK = D // 128               # 8 contraction tiles for projections


def _lead(cls):
    return cls[1] * KT_W if isinstance(cls, tuple) and cls[0] == "tri" else 0


def _build_nc(block_class, n_uniq, repeat=1, loop_n=0, parts=('qk','v','sc','exp','mask','pv','out')):
    """block_class: dict[(qc,kt)] -> 'f' | ('tri', j) | ('m', idx).
    Fully-masked blocks are absent. Same program for all cores (SPMD)."""
    nc = bacc.Bacc("TRN2", target_bir_lowering=False, debug=False, num_devices=NCORES)

    f32, bf16 = dt.float32, dt.bfloat16
    t_xq = nc.dram_tensor("xq", [D, S], bf16, kind="ExternalInput").ap()
    t_xk = nc.dram_tensor("xk", [D, S], bf16, kind="ExternalInput").ap()
    t_xv = nc.dram_tensor("xv", [D, S], bf16, kind="ExternalInput").ap()
    t_wq = nc.dram_tensor("wq", [D, E], bf16, kind="ExternalInput").ap()
    t_wk = nc.dram_tensor("wk", [D, E], bf16, kind="ExternalInput").ap()
    t_wv = nc.dram_tensor("wv", [D, E], bf16, kind="ExternalInput").ap()
    t_wo = nc.dram_tensor("wo", [E, D], bf16, kind="ExternalInput").ap()
    t_bq = nc.dram_tensor("bq", [128, 2], f32, kind="ExternalInput").ap()
    t_bk = nc.dram_tensor("bk", [128, 2], f32, kind="ExternalInput").ap()
    t_bv = nc.dram_tensor("bv", [1, E], f32, kind="ExternalInput").ap()
    t_m01 = None
    if n_uniq:
        t_m01 = nc.dram_tensor(
            "m01", [n_uniq, KT_W, QC_W], bf16, kind="ExternalInput"
        ).ap()
    has_tri = any(isinstance(c, tuple) and c[0] == "tri" for c in block_class.values())
    t_tri = None
    if has_tri:
        t_tri = nc.dram_tensor("tri", [KT_W, KT_W], bf16, kind="ExternalInput").ap()
    t_out = nc.dram_tensor("out", [S, D], bf16, kind="ExternalOutput").ap()

    with tile.TileContext(nc) as tc, ExitStack() as ctx:
        singles = ctx.enter_context(tc.tile_pool(name="singles", bufs=1))

        # --- resident weights / biases / mask tiles (loaded once) ---
        wq_sb = singles.tile([128, NXK, E], bf16, name="wq_sb")
        wk_sb = singles.tile([128, NXK, E], bf16, name="wk_sb")
        wv_sb = singles.tile([128, NXK, E], bf16, name="wv_sb")
        wo_sb = singles.tile([128, 2, D], bf16, name="wo_sb")
        bq_sb = singles.tile([128, 2], f32, name="bq_sb")
        bk_sb = singles.tile([128, 2], f32, name="bk_sb")
        bv_sb = singles.tile([128, G, DK], f32, name="bv_sb")
        nc.sync.dma_start(out=wq_sb, in_=t_wq.rearrange("(k p) e -> p k e", p=128))
        nc.sync.dma_start(out=bq_sb, in_=t_bq)
        nc.sync.dma_start(out=wk_sb, in_=t_wk.rearrange("(k p) e -> p k e", p=128))
        nc.sync.dma_start(out=bk_sb, in_=t_bk)
        nc.sync.dma_start(out=wv_sb, in_=t_wv.rearrange("(k p) e -> p k e", p=128))
        nc.sync.dma_start(
            out=bv_sb, in_=t_bv.rearrange("o (h d) -> o h d", d=DK).to_broadcast([128, G, DK])
        )
        nc.sync.dma_start(out=wo_sb, in_=t_wo.rearrange("(k p) e -> p k e", p=128))

        tri_sb = None
        if has_tri:
            tri_sb = singles.tile([KT_W, KT_W], bf16, name="tri_sb")
            nc.sync.dma_start(out=tri_sb, in_=t_tri)

        # --- persistent activations ---
        act_sb = ctx.enter_context(tc.tile_pool(name="act_sb", bufs=1))
        qt_sb = [act_sb.tile([128, S], bf16, name=f"qt_sb{i}") for i in range(2)]
        kt_sb = [act_sb.tile([128, S], bf16, name=f"kt_sb{i}") for i in range(2)]
        # V with 64 replicated ones columns per head: [s-tile 128, head, 2*dk]
        v_sb = [act_sb.tile([128, G, 2 * DK], bf16, name=f"v_sb{i}") for i in range(NMS)]
        xt_sb = [act_sb.tile([128, S], bf16, name=f"xt_sb{i}") for i in range(2)]
        for ms in range(NMS):
            nc.vector.memset(v_sb[ms][:, :, DK : 2 * DK], 1.0)

        import contextlib

        for rep in range(repeat):
            sfx = f"r{rep}"
            loop_cm = tc.For_i(0, loop_n, 1) if loop_n else contextlib.nullcontext()
            ctx2 = ExitStack()
            ctx2.enter_context(loop_cm)

            # ---------- input prefetch: all x DMAs issued up front ----------
            # Issuing every input DMA at body start keeps the SP HWDGE ring
            # busy back-to-back (xq chunks, then xk, then xv) instead of
            # serializing each load behind the previous phase's pool close.
            xv_pool = ctx2.enter_context(tc.tile_pool(name=f"xv{sfx}", bufs=1))
            xv_t = xv_pool.tile([128, NXK, S], bf16, name=f"x_v{sfx}", tag="xv")
            xqk_pool = ExitStack()
            xqk = xqk_pool.enter_context(tc.tile_pool(name=f"xqk{sfx}", bufs=1))
            xq_t = [
                xqk.tile([128, NXK // 2, S], bf16, name=f"x_q{sfx}_{c}", tag=f"xq{c}")
                for c in range(2)
            ]
            xk_t = xqk.tile([128, NXK, S], bf16, name=f"x_k{sfx}", tag="xk")
            if 'qk' in parts:
                for c in range(2):
                    nc.sync.dma_start(
                        out=xq_t[c],
                        in_=t_xq.rearrange("(c k p) s -> c p k s", c=2, p=128)[c],
                    )
                nc.sync.dma_start(
                    out=xk_t, in_=t_xk.rearrange("(k p) s -> p k s", p=128)
                )
            if 'v' in parts:
                nc.sync.dma_start(
                    out=xv_t, in_=t_xv.rearrange("(k p) s -> p k s", p=128)
                )

            # ---------- Q^T / K^T projections, contraction-outer ----------
            for pname, xts, w_sb, b_sb, o_sb in ((
                ("q", [xq_t[0][:, k] for k in range(4)] + [xq_t[1][:, k] for k in range(4)],
                 wq_sb, bq_sb, qt_sb),
                ("k", [xk_t[:, k] for k in range(NXK)], wk_sb, bk_sb, kt_sb),
            ) if 'qk' in parts else ()):
                with tc.tile_pool(name=f"ps{pname}{sfx}", bufs=1, space="PSUM") as pp:
                    ps = [
                        pp.tile([128, QC_W], f32, name=f"ps_{pname}{sfx}_{i}", tag=f"ps{i}")
                        for i in range(8)
                    ]
                    for kt in range(NXK):
                        for nc4 in range(NQC):
                            for mt in range(2):
                                nc.tensor.matmul(
                                    ps[nc4 * 2 + mt],
                                    w_sb[:, kt, mt * 128 : (mt + 1) * 128],
                                    xts[kt][:, nc4 * QC_W : (nc4 + 1) * QC_W],
                                    start=(kt == 0),
                                    stop=(kt == NXK - 1),
                                )
                    # evictions alternate DVE/ScalarE (ScalarE idle pre-scores)
                    for nc4 in range(NQC):
                        for mt in range(2):
                            osl = o_sb[mt][:, nc4 * QC_W : (nc4 + 1) * QC_W]
                            if (nc4 * 2 + mt) % 2 == 0:
                                nc.vector.tensor_scalar_add(
                                    osl, ps[nc4 * 2 + mt], b_sb[:, mt : mt + 1]
                                )
                            else:
                                nc.scalar.activation(
                                    osl, ps[nc4 * 2 + mt], AFT.Identity,
                                    bias=b_sb[:, mt : mt + 1],
                                )

            xqk_pool.close()

            # ---------- attention + V projection + output projection ----------
            # Two instruction streams woven by estimated cost so PE and
            # ScalarE stay concurrently busy despite in-order engine queues:
            #   stream 1: score matmuls + exp + mask (exp-bound, 1 step/st-tile)
            #   stream 2: V-projection blocks, PV accumulation, out-projection
            # A stream-2 item is issued once enough exp work is in flight to
            # cover its PE time (plus explicit min-step dependency gates).
            with (
                tc.tile_pool(name=f"pt{sfx}", bufs=34) as pt_pool,
                tc.tile_pool(name=f"xa{sfx}", bufs=2, space="PSUM") as xa_psum,
                tc.tile_pool(name=f"rec{sfx}", bufs=2) as rec_pool,
                tc.tile_pool(name=f"ot{sfx}", bufs=2) as ot_pool,
            ):
                xv_sb = [xv_t[:, kt] for kt in range(NXK)]

                kts_of = {
                    qc: [kt for kt in range(NKT) if (qc, kt) in block_class]
                    for qc in range(NQC)
                }
                pts = {}  # (qc, pr, kt, lh) -> (pt tile, h offset index)

                # ----- stream 1: score matmul + exp + mask steps -----
                score_steps = []  # (emit_fn, exp_cost_ns)
                sc_done = {}  # (qc, pr) -> score step index after which done

                def _score_step(qc, pr, pair, lh):
                    def emit():
                        w = len(pair) * QC_W
                        st = st_psum.tile(
                            [128, 2 * QC_W], f32,
                            name=f"st{sfx}_{pr}_{qc}_{pair[0]}_{lh}", tag="st",
                        )
                        pt = pt_pool.tile(
                            [128, 2 * QC_W], bf16,
                            name=f"pt{sfx}_{pr}_{qc}_{pair[0]}_{lh}", tag="pt",
                        )
                        for h, kt in enumerate(pair) if 'sc' in parts else ():
                            lead = _lead(block_class[(qc, kt)])
                            nc.tensor.matmul(
                                st[:, h * QC_W + lead : (h + 1) * QC_W],
                                kt_sb[pr][
                                    lh * DK : (lh + 1) * DK,
                                    kt * KT_W : (kt + 1) * KT_W,
                                ],
                                qt_sb[pr][
                                    lh * DK : (lh + 1) * DK,
                                    qc * QC_W + lead : (qc + 1) * QC_W,
                                ],
                            )
                            pts[(qc, pr, kt, lh)] = (pt, h)
                        if 'exp' in parts:
                            for h, kt in enumerate(pair):
                                lead = _lead(block_class[(qc, kt)])
                                nc.scalar.activation(
                                    pt[:, h * QC_W + lead : (h + 1) * QC_W],
                                    st[:, h * QC_W + lead : (h + 1) * QC_W],
                                    AFT.Exp,
                                )
                        for h, kt in enumerate(pair) if 'mask' in parts else ():
                            cls = block_class[(qc, kt)]
                            off = h * QC_W
                            if isinstance(cls, tuple) and cls[0] == "tri":
                                j = cls[1]
                                nc.gpsimd.tensor_mul(
                                    pt[:, off + j * KT_W : off + (j + 1) * KT_W],
                                    pt[:, off + j * KT_W : off + (j + 1) * KT_W],
                                    tri_sb,
                                )
                            elif isinstance(cls, tuple) and cls[0] == "m":
                                mt_ = pt_pool.tile(
                                    [KT_W, QC_W], bf16,
                                    name=f"m01u{sfx}_{pr}_{qc}_{kt}_{lh}",
                                    tag="m01u", bufs=4,
                                )
                                nc.sync.dma_start(out=mt_, in_=t_m01[cls[1]])
                                nc.gpsimd.tensor_mul(
                                    pt[:, off : off + QC_W],
                                    pt[:, off : off + QC_W],
                                    mt_,
                                )

                    cols = sum(QC_W - _lead(block_class[(qc, kt)]) for kt in pair)
                    return emit, cols * 1.5 + 200.0

                for qc in range(NQC):
                    for pr in range(2):
                        kts = kts_of[qc]
                        for i in range(0, len(kts), 2):
                            pair = tuple(kts[i : i + 2])
                            for lh in range(2):
                                score_steps.append(_score_step(qc, pr, pair, lh))
                        sc_done[(qc, pr)] = len(score_steps)

                # ----- stream 2: V blocks, PV+normalize halves, out-proj -----
                def _v_item(t):
                    def emit():
                        vp = v_psum.tile(
                            [128, 2, G, DK], f32, name=f"ps_v{sfx}_{t}", tag="psv"
                        )
                        # one accumulation group at a time per PSUM bank
                        for m in range(2):
                            for kt in range(NXK):
                                nc.tensor.matmul(
                                    vp[:, m],
                                    xv_sb[kt][
                                        :, (2 * t + m) * 128 : (2 * t + m + 1) * 128
                                    ],
                                    wv_sb[:, kt, :],
                                    start=(kt == 0),
                                    stop=(kt == NXK - 1),
                                )
                        for m in range(2):
                            nc.vector.tensor_add(
                                v_sb[2 * t + m][:, :, 0:DK], vp[:, m], bv_sb
                            )

                    return emit, 2050.0, (12 if t < 2 else 0)

                def _pv_item(qc, pr, lh):
                    kts = kts_of[qc]

                    def emit():
                        xa = xa_psum.tile(
                            [128, QC_W], f32, name=f"xa{sfx}_{pr}_{qc}_{lh}", tag="xa"
                        )
                        for i, kt in enumerate(kts):
                            lead = _lead(block_class[(qc, kt)])
                            pt, h = pts[(qc, pr, kt, lh)]
                            nc.tensor.matmul(
                                xa[:, lead:QC_W],
                                v_sb[kt][:, pr * 2 + lh, :],
                                pt[:, h * QC_W + lead : (h + 1) * QC_W],
                                start=(i == 0),
                                stop=(i == len(kts) - 1),
                            )
                        rec = rec_pool.tile(
                            [DK, 2, QC_W], f32, name=f"rec{sfx}_{pr}_{qc}_{lh}", tag="rec"
                        )
                        nc.vector.tensor_copy(rec[:, 1], xa[DK : 2 * DK, :])
                        nc.vector.reciprocal_approx_fast(rec[:, 0], rec[:, 1])
                        rec = rec[:, 0]
                        nc.vector.tensor_mul(
                            xt_sb[pr][
                                lh * DK : (lh + 1) * DK, qc * QC_W : (qc + 1) * QC_W
                            ],
                            xa[0:DK, :],
                            rec,
                        )

                    pe = sum(
                        (QC_W - _lead(block_class[(qc, kt)])) * 0.5 for kt in kts
                    )
                    return emit, pe, sc_done[(qc, pr)]

                ot_stage = {}

                def _o_item(qc, msi):
                    def emit():
                        ms = qc * 4 + msi
                        if msi == 0:
                            ot_stage[qc] = ot_pool.tile(
                                [128, 4, 2, QC_W], bf16, name=f"ot{sfx}_{qc}", tag="ot"
                            )
                        ot = ot_stage[qc]
                        for nc2 in range(2):
                            o = o_psum.tile(
                                [128, QC_W], f32, name=f"o{sfx}_{ms}_{nc2}", tag="o"
                            )
                            for pr in range(2):
                                nc.tensor.matmul(
                                    o,
                                    xt_sb[pr][:, ms * 128 : (ms + 1) * 128],
                                    wo_sb[:, pr, nc2 * QC_W : (nc2 + 1) * QC_W],
                                    start=(pr == 0),
                                    stop=(pr == 1),
                                )
                            nc.vector.tensor_copy(ot[:, msi, nc2], o)
                        if msi == 3:
                            nc.sync.dma_start(
                                out=t_out[
                                    qc * QC_W : (qc + 1) * QC_W, :
                                ].rearrange("(m p) d -> p m d", p=128),
                                in_=ot.rearrange("p m n w -> p m (n w)"),
                            )

                    return emit, 1000.0, sc_done[(qc, 1)]

                _n = lambda: (lambda: None, 0.0, 0)
                _v_item_ = _v_item if 'v' in parts else lambda t: _n()
                _pv_item_ = _pv_item if 'pv' in parts else lambda qc, pr, lh: _n()
                _o_item_ = _o_item if 'out' in parts else lambda qc, m: _n()
                _v_item, _pv_item, _o_item = _v_item_, _pv_item_, _o_item_
                stream2a = [_v_item(0), _v_item(1)]
                stream2a += [_pv_item(0, pr, lh) for pr in range(2) for lh in range(2)]
                stream2a += [_v_item(2), _v_item(3)]
                stream2a += [_pv_item(1, pr, lh) for pr in range(2) for lh in range(2)]
                stream2a += [_v_item(t) for t in range(4, NXK)]
                stream2b = [_o_item(0, m) for m in range(4)]
                stream2b += [_pv_item(2, pr, lh) for pr in range(2) for lh in range(2)]
                stream2b += [_o_item(1, m) for m in range(4)]
                stream2b += [_pv_item(3, pr, lh) for pr in range(2) for lh in range(2)]
                stream2b += [_o_item(2, m) for m in range(4)]
                stream2b += [_o_item(3, m) for m in range(4)]

                state = {"si": 0, "t_exp": 0.0, "t2": 0.0}

                def weave(stream2):
                    for emit2, pe2, min_s in stream2:
                        while state["si"] < len(score_steps) and (
                            state["si"] < min_s
                            or state["t2"] + pe2 > state["t_exp"]
                        ):
                            emit, e = score_steps[state["si"]]
                            emit()
                            state["t_exp"] += e
                            state["si"] += 1
                        emit2()
                        state["t2"] += pe2
                    return

                with tc.tile_pool(name=f"st{sfx}", bufs=2, space="PSUM") as st_psum:
                    with tc.tile_pool(
                        name=f"v_ps{sfx}", bufs=2, space="PSUM"
                    ) as v_psum:
                        weave(stream2a)
                    with tc.tile_pool(
                        name=f"o_ps{sfx}", bufs=2, space="PSUM"
                    ) as o_psum:
                        weave(stream2b)
                        while state["si"] < len(score_steps):
                            emit, e = score_steps[state["si"]]
                            emit()
                            state["si"] += 1

            ctx2.close()

    nc.compile()
    return nc


def _classify(mask):
    """Block classification shared by all cores + per-core unique mask tiles.

    Returns (block_class, n_uniq, per_batch_m01) where block_class maps
    (qc, kt) -> 'f' | ('tri', j) | ('m', uniq-index); fully-masked-everywhere
    blocks are absent. per_batch_m01[b] is an [n_uniq, 128, 512] bf16 array.
    """
    mask = np.asarray(mask)
    blk = mask.reshape(B, NQC, QC_W, NKT, KT_W)
    nz = (blk != 0).sum(axis=(2, 4))  # [B, NQC, NKT]
    full = nz == QC_W * KT_W
    empty = nz == 0

    block_class = {}
    uniq = {}
    per_batch = [[] for _ in range(B)]
    qs = np.arange(QC_W)
    ks = np.arange(KT_W)
    for qc in range(NQC):
        for kt in range(NKT):
            if empty[:, qc, kt].all():
                continue
            if full[:, qc, kt].all():
                block_class[(qc, kt)] = "f"
                continue
            blocks = [
                (mask[b, qc * QC_W : (qc + 1) * QC_W, kt * KT_W : (kt + 1) * KT_W] != 0)
                for b in range(B)
            ]
            # causal-shaped block? pattern q >= k with aligned j offset
            j, rem = divmod(kt * KT_W - qc * QC_W, KT_W)
            if rem == 0 and 0 <= j and (j + 1) * KT_W <= QC_W:
                pat = (qc * QC_W + qs)[:, None] >= (kt * KT_W + ks)[None, :]
                if all((blk == pat).all() for blk in blocks):
                    block_class[(qc, kt)] = ("tri", j)
                    continue
            subs = [np.ascontiguousarray(blk.T).astype(BF16) for blk in blocks]
            key = b"".join(s.tobytes() for s in subs)
            if key not in uniq:
                uniq[key] = len(uniq)
                for b in range(B):
                    per_batch[b].append(subs[b])
            block_class[(qc, kt)] = ("m", uniq[key])
    n_uniq = len(uniq)
    m01 = [
        np.stack(per_batch[b]) if n_uniq else None
        for b in range(B)
    ]
    return block_class, n_uniq, m01


def _prep_inputs(query, key, value, mask, Wq, bq, Wk, bk, Wv, bv, Wo, bo):
    """Returns (in_maps, block_class, n_uniq)."""
    block_class, n_uniq, m01 = _classify(mask)
    scale = 1.0 / np.sqrt(np.float32(DK))

    xq = [np.ascontiguousarray(np.asarray(query[b]).T).astype(BF16) for b in range(B)]
    xk = [np.ascontiguousarray(np.asarray(key[b]).T).astype(BF16) for b in range(B)]
    xv = [np.ascontiguousarray(np.asarray(value[b]).T).astype(BF16) for b in range(B)]

    Wq, Wk, Wv, Wo = (np.asarray(a, np.float32) for a in (Wq, Wk, Wv, Wo))
    bq, bk, bv = (np.asarray(a, np.float32) for a in (bq, bk, bv))

    in_maps = []
    for c in range(NCORES):
        b, g = divmod(c, G)
        sl = slice(g * E, (g + 1) * E)
        im = {
            "xq": xq[b],
            "xk": xk[b],
            "xv": xv[b],
            "wq": np.ascontiguousarray(Wq[sl].T * scale).astype(BF16),
            "wk": np.ascontiguousarray(Wk[sl].T).astype(BF16),
            "wv": np.ascontiguousarray(Wv[sl].T).astype(BF16),
            "wo": np.ascontiguousarray(Wo[:, sl].T).astype(BF16),
            "bq": np.ascontiguousarray((bq[sl] * scale).reshape(2, 128).T),
            "bk": np.ascontiguousarray(bk[sl].reshape(2, 128).T),
            "bv": np.ascontiguousarray(bv[sl][None, :]),
        }
        if n_uniq:
            im["m01"] = m01[b]
        if any(isinstance(c2, tuple) and c2[0] == "tri" for c2 in block_class.values()):
            p = np.arange(KT_W)
            im["tri"] = np.ascontiguousarray(
                (p[None, :] >= p[:, None])
            ).astype(BF16)
        in_maps.append(im)
    return in_maps, block_class, n_uniq


_NC_CACHE = {}


def _get_nc(block_class, n_uniq, repeat=1):
    key = (tuple(sorted(block_class.items())), n_uniq, repeat)
    if key not in _NC_CACHE:
        _NC_CACHE[key] = _build_nc(block_class, n_uniq, repeat=repeat)
    return _NC_CACHE[key]


def kernel(query, key, value, mask, Wq, bq, Wk, bk, Wv, bv, Wo, bo):
    in_maps, block_class, n_uniq = _prep_inputs(
        query, key, value, mask, Wq, bq, Wk, bk, Wv, bv, Wo, bo
    )
    nc = _get_nc(block_class, n_uniq)
    res = run_bass_kernel_spmd(nc, in_maps, core_ids=list(range(NCORES)))
    bo = np.asarray(bo, np.float32)
    out = np.empty((B, S, D), np.float32)
    for b in range(B):
        acc = res.results[b * G]["out"].astype(np.float32)
        for g in range(1, G):
            acc = acc + res.results[b * G + g]["out"]
        out[b] = acc + bo[None, :]
    return out



# revision 15
# speedup vs baseline: 1.0363x; 1.0363x over previous
"""Trainium2 Bass kernel for nn_MultiHeadedAttention (B=2, S=2048, D=1024, H=16).

Sharding (8 NeuronCores): tensor-parallel over heads x data-parallel over batch.
Core c handles batch b = c // 4 and head group g = c % 4 (4 heads = 256 dims).

Per-core pipeline (all matmuls bf16, fp32 PSUM accumulation):
  - Inputs streamed in fine chunks: xq (4) + xk (2) on the SP HWDGE ring,
    xv (4 s-chunks) + output stores on the ACT HWDGE ring, so the two rings
    drain in parallel and each consumer starts on its first chunk.
  - Q^T/K^T projections in transposed layout [e, s], contraction(kt)-OUTER so
    matmuls start as soon as the first x-tile DMA lands. Q and K share one
    8-tile PSUM pool (per-tag handoff: K's slot-i matmuls wait only on Q's
    slot-i eviction). Per-partition bias via DVE/ScalarE on eviction;
    1/sqrt(dk) folded into Wq host-side.
  - Scores computed transposed, S^T = K @ Q^T, per (512-q-chunk, 128-k-tile)
    with causally dead leading columns trimmed off the matmul and PV reads.
    The two head-halves (lh) of each step land on PE row-groups 0/64 via
    implicit tile_position, enabling hardware row-tiling concurrency.
    exp on ScalarE straight out of PSUM (scores bounded, no max-subtraction),
    merged into one [128, 1024] instruction per step when the wasted lead
    region is small. Diagonal (tri) pairs are emitted first within each
    (qc, pr) so their GPSIMD masks clear the PV dependency chain early.
  - Mask applied as data on GPSIMD (SBUF-only engine): adjacent-j tri pairs
    masked with a single 2-region strided multiply.
  - V projection [s, e] augmented with 64 REPLICATED ones columns per head:
    the PV matmul then emits the denominator replicated across partitions
    64-127, so normalization is a plain [64,512] reciprocal + multiply on DVE.
    V tiles are allocated per kt-PAIR so the two bias adds merge into one.
  - PV accumulates X^T per (q-chunk, head) with V_aug stationary.
  - Output projection per q-chunk interleaved behind attention; evictions on
    DVE; output DMA issued from the ACT ring; host reduces the 4 head-group
    partials per batch + bo.
"""

from contextlib import ExitStack

import numpy as np
import ml_dtypes

import concourse.bass as bass  # noqa: F401
import concourse.bacc as bacc
import concourse.tile as tile
import concourse.mybir as mybir
from concourse.bass_utils import run_bass_kernel_spmd

dt = mybir.dt
AFT = mybir.ActivationFunctionType
BF16 = ml_dtypes.bfloat16

B, S, D, H = 2, 2048, 1024, 16
DK = D // H                  # 64
NCORES = 8
G = 4                        # heads per core
E = G * DK                   # 256 projected dims per core
QC_W = 512                   # q chunk width
KT_W = 128                   # k tile width
NQC = S // QC_W              # 4
NKT = S // KT_W              # 16
NMS = S // 128               # 16 s-tiles
NXK = D // 128               # 8 contraction tiles for projections
NVP = NMS // 2               # 8 v s-tile-pair tiles


def _lead(cls):
    return cls[1] * KT_W if isinstance(cls, tuple) and cls[0] == "tri" else 0


def _build_nc(block_class, n_uniq, repeat=1, loop_n=0, parts=('qk','v','sc','exp','mask','pv','out')):
    """block_class: dict[(qc,kt)] -> 'f' | ('tri', j) | ('m', idx).
    Fully-masked blocks are absent. Same program for all cores (SPMD)."""
    nc = bacc.Bacc("TRN2", target_bir_lowering=False, debug=False, num_devices=NCORES)

    f32, bf16 = dt.float32, dt.bfloat16
    t_xq = nc.dram_tensor("xq", [D, S], bf16, kind="ExternalInput").ap()
    t_xk = nc.dram_tensor("xk", [D, S], bf16, kind="ExternalInput").ap()
    t_xv = nc.dram_tensor("xv", [D, S], bf16, kind="ExternalInput").ap()
    t_wq = nc.dram_tensor("wq", [D, E], bf16, kind="ExternalInput").ap()
    t_wk = nc.dram_tensor("wk", [D, E], bf16, kind="ExternalInput").ap()
    t_wv = nc.dram_tensor("wv", [D, E], bf16, kind="ExternalInput").ap()
    t_wo = nc.dram_tensor("wo", [E, D], bf16, kind="ExternalInput").ap()
    t_bq = nc.dram_tensor("bq", [128, 2], f32, kind="ExternalInput").ap()
    t_bk = nc.dram_tensor("bk", [128, 2], f32, kind="ExternalInput").ap()
    t_bv = nc.dram_tensor("bv", [1, E], f32, kind="ExternalInput").ap()
    t_m01 = None
    if n_uniq:
        t_m01 = nc.dram_tensor(
            "m01", [n_uniq, KT_W, QC_W], bf16, kind="ExternalInput"
        ).ap()
    has_tri = any(isinstance(c, tuple) and c[0] == "tri" for c in block_class.values())
    t_tri = None
    if has_tri:
        t_tri = nc.dram_tensor("tri", [KT_W, KT_W], bf16, kind="ExternalInput").ap()
    t_out = nc.dram_tensor("out", [S, D], bf16, kind="ExternalOutput").ap()

    with tile.TileContext(nc) as tc, ExitStack() as ctx:
        singles = ctx.enter_context(tc.tile_pool(name="singles", bufs=1))

        # --- resident weights / biases / mask tiles (loaded once) ---
        wq_sb = singles.tile([128, NXK, E], bf16, name="wq_sb")
        wk_sb = singles.tile([128, NXK, E], bf16, name="wk_sb")
        wv_sb = singles.tile([128, NXK, E], bf16, name="wv_sb")
        wo_sb = singles.tile([128, 2, D], bf16, name="wo_sb")
        bq_sb = singles.tile([128, 2], f32, name="bq_sb")
        bk_sb = singles.tile([128, 2], f32, name="bk_sb")
        bv_sb = singles.tile([128, G, DK], f32, name="bv_sb")
        # wq/bq on the SP ring ahead of the xq stream; the rest on the ACT
        # ring so the first Q-projection matmul starts as early as possible.
        nc.sync.dma_start(out=wq_sb, in_=t_wq.rearrange("(k p) e -> p k e", p=128))
        nc.sync.dma_start(out=bq_sb, in_=t_bq)
        nc.scalar.dma_start(out=wk_sb, in_=t_wk.rearrange("(k p) e -> p k e", p=128))
        nc.scalar.dma_start(out=bk_sb, in_=t_bk)
        nc.scalar.dma_start(out=wv_sb, in_=t_wv.rearrange("(k p) e -> p k e", p=128))
        nc.scalar.dma_start(
            out=bv_sb, in_=t_bv.rearrange("o (h d) -> o h d", d=DK).to_broadcast([128, G, DK])
        )
        nc.scalar.dma_start(out=wo_sb, in_=t_wo.rearrange("(k p) e -> p k e", p=128))

        tri_sb = None
        if has_tri:
            tri_sb = singles.tile([KT_W, KT_W], bf16, name="tri_sb")
            nc.scalar.dma_start(out=tri_sb, in_=t_tri)

        # --- persistent activations ---
        act_sb = ctx.enter_context(tc.tile_pool(name="act_sb", bufs=1))
        qt_sb = [act_sb.tile([128, S], bf16, name=f"qt_sb{i}") for i in range(2)]
        kt_sb = [act_sb.tile([128, S], bf16, name=f"kt_sb{i}") for i in range(2)]
        # V with 64 replicated ones columns per head, allocated per kt-PAIR:
        # [s-tile 128, pair-slot 2, head, 2*dk]
        v2_sb = [
            act_sb.tile([128, 2, G, 2 * DK], bf16, name=f"v2_sb{i}")
            for i in range(NVP)
        ]
        xt_sb = [act_sb.tile([128, S], bf16, name=f"xt_sb{i}") for i in range(2)]
        for vp_ in range(NVP):
            nc.vector.memset(v2_sb[vp_][:, :, :, DK : 2 * DK], 1.0)

        def v_sb(kt):
            return v2_sb[kt // 2][:, kt % 2]

        import contextlib

        loop_cm = tc.For_i(0, loop_n, 1) if loop_n else contextlib.nullcontext()
        loop_ctx = ExitStack()
        loop_ctx.enter_context(loop_cm)
        for rep in range(repeat):
            sfx = f"r{rep}"
            ctx2 = ExitStack()

            # ---------- input prefetch ----------
            # xq/xk stream on the SP HWDGE ring in fine chunks (projection
            # matmuls start on chunk 0); xv takes the ACT ring in s-chunks so
            # the first V-projection block only waits on its own chunk.
            xv_pool = ctx2.enter_context(tc.tile_pool(name=f"xv{sfx}", bufs=1))
            xv_c = [
                xv_pool.tile([128, NXK, QC_W], bf16, name=f"x_v{sfx}_{c}", tag=f"xv{c}")
                for c in range(4)
            ]
            xqk_pool = ExitStack()
            xqk = xqk_pool.enter_context(tc.tile_pool(name=f"xqk{sfx}", bufs=1))
            xq_c = [
                xqk.tile([128, 2, S], bf16, name=f"x_q{sfx}_{c}", tag=f"xq{c}")
                for c in range(4)
            ]
            xk_c = [
                xqk.tile([128, 4, S], bf16, name=f"x_k{sfx}_{c}", tag=f"xk{c}")
                for c in range(2)
            ]
            if 'qk' in parts:
                xq_r = t_xq.rearrange("(c k p) s -> c p k s", c=4, p=128)
                xk_r = t_xk.rearrange("(c k p) s -> c p k s", c=2, p=128)
                # SP ring order: xq0 xq1 xk0 xq2 xq3 xk1 then xv — Q-proj is
                # paced by xq chunks; xk0 lands before K-proj starts; xv
                # streams behind (first V block isn't needed until the
                # attention weave is underway).
                nc.sync.dma_start(out=xq_c[0], in_=xq_r[0])
                nc.sync.dma_start(out=xq_c[1], in_=xq_r[1])
                nc.sync.dma_start(out=xk_c[0], in_=xk_r[0])
                nc.sync.dma_start(out=xq_c[2], in_=xq_r[2])
                nc.sync.dma_start(out=xq_c[3], in_=xq_r[3])
                nc.sync.dma_start(out=xk_c[1], in_=xk_r[1])
            if 'v' in parts:
                xv_r = t_xv.rearrange("(k p) s -> p k s", p=128)
                for c in range(4):
                    nc.sync.dma_start(
                        out=xv_c[c], in_=xv_r[:, :, c * QC_W : (c + 1) * QC_W]
                    )

            def xq_t(kt):
                return xq_c[kt // 2][:, kt % 2]

            def xk_t(kt):
                return xk_c[kt // 4][:, kt % 4]

            # ---------- Q^T / K^T projections, contraction-outer ----------
            # One shared 8-slot PSUM pool: K's slot-i matmuls depend only on
            # Q's slot-i eviction (no all-slot barrier between projections).
            with tc.tile_pool(name=f"pqk{sfx}", bufs=1, space="PSUM") as pp:
                for pname, xts, w_sb, b_sb, o_sb in ((
                    ("q", [xq_t(k) for k in range(NXK)], wq_sb, bq_sb, qt_sb),
                    ("k", [xk_t(k) for k in range(NXK)], wk_sb, bk_sb, kt_sb),
                ) if 'qk' in parts else ()):
                    ps = [
                        pp.tile([128, QC_W], f32, name=f"ps_{pname}{sfx}_{i}", tag=f"ps{i}")
                        for i in range(8)
                    ]
                    for kt in range(NXK):
                        for nc4 in range(NQC):
                            for mt in range(2):
                                nc.tensor.matmul(
                                    ps[nc4 * 2 + mt],
                                    w_sb[:, kt, mt * 128 : (mt + 1) * 128],
                                    xts[kt][:, nc4 * QC_W : (nc4 + 1) * QC_W],
                                    start=(kt == 0),
                                    stop=(kt == NXK - 1),
                                )
                    # evictions alternate DVE/ScalarE (ScalarE idle pre-scores)
                    for nc4 in range(NQC):
                        for mt in range(2):
                            osl = o_sb[mt][:, nc4 * QC_W : (nc4 + 1) * QC_W]
                            if (nc4 * 2 + mt) % 2 == 0:
                                nc.vector.tensor_scalar_add(
                                    osl, ps[nc4 * 2 + mt], b_sb[:, mt : mt + 1]
                                )
                            else:
                                nc.scalar.activation(
                                    osl, ps[nc4 * 2 + mt], AFT.Identity,
                                    bias=b_sb[:, mt : mt + 1],
                                )

            xqk_pool.close()

            # ---------- attention + V projection + output projection ----------
            # Two instruction streams woven by estimated cost so PE and
            # ScalarE stay concurrently busy despite in-order engine queues:
            #   stream 1: score matmuls + exp + mask (exp-bound, 1 step/st-tile)
            #   stream 2: V-projection blocks, PV accumulation, out-projection
            # A stream-2 item is issued once enough exp work is in flight to
            # cover its PE time (plus explicit min-step dependency gates).
            with (
                tc.tile_pool(name=f"pt{sfx}", bufs=40) as pt_pool,
                tc.tile_pool(name=f"xa{sfx}", bufs=2, space="PSUM") as xa_psum,
                tc.tile_pool(name=f"rec{sfx}", bufs=2) as rec_pool,
                tc.tile_pool(name=f"ot{sfx}", bufs=2) as ot_pool,
            ):
                kts_of = {
                    qc: [kt for kt in range(NKT) if (qc, kt) in block_class]
                    for qc in range(NQC)
                }
                pts = {}  # (qc, pr, kt, lh) -> (pt tile, h offset index)

                # ----- stream 1: score matmul + exp + mask steps -----
                score_steps = []  # (emit_fn, exp_cost_ns)
                sc_done = {}  # (qc, pr) -> score step index after which done

                def _score_step(qc, pr, pair, lh):
                    leads = [_lead(block_class[(qc, kt)]) for kt in pair]

                    def emit():
                        st = st_psum.tile(
                            [128, 2 * QC_W], f32,
                            name=f"st{sfx}_{pr}_{qc}_{pair[0]}_{lh}", tag="st",
                        )
                        pt = pt_pool.tile(
                            [128, 2 * QC_W], bf16,
                            name=f"pt{sfx}_{pr}_{qc}_{pair[0]}_{lh}", tag="pt",
                        )
                        for h, kt in enumerate(pair) if 'sc' in parts else ():
                            lead = leads[h]
                            nc.tensor.matmul(
                                st[:, h * QC_W + lead : (h + 1) * QC_W],
                                kt_sb[pr][
                                    lh * DK : (lh + 1) * DK,
                                    kt * KT_W : (kt + 1) * KT_W,
                                ],
                                qt_sb[pr][
                                    lh * DK : (lh + 1) * DK,
                                    qc * QC_W + lead : (qc + 1) * QC_W,
                                ],
                            )
                            pts[(qc, pr, kt, lh)] = (pt, h)
                        if 'exp' in parts:
                            # One merged exp when the unwritten lead region is
                            # small (reading stale PSUM there is safe: those
                            # columns are never consumed downstream).
                            if len(pair) == 2 and sum(leads) <= KT_W:
                                nc.scalar.activation(
                                    pt[:, leads[0] :],
                                    st[:, leads[0] :],
                                    AFT.Exp,
                                )
                            else:
                                for h in range(len(pair)):
                                    nc.scalar.activation(
                                        pt[:, h * QC_W + leads[h] : (h + 1) * QC_W],
                                        st[:, h * QC_W + leads[h] : (h + 1) * QC_W],
                                        AFT.Exp,
                                    )
                        if 'mask' in parts:
                            tris = [
                                (h, block_class[(qc, kt)][1])
                                for h, kt in enumerate(pair)
                                if isinstance(block_class[(qc, kt)], tuple)
                                and block_class[(qc, kt)][0] == "tri"
                            ]
                            if (
                                len(tris) == 2
                                and tris[0][1] + 1 == tris[1][1]
                            ):
                                # Adjacent-j pair: both diagonal blocks in one
                                # 2-region strided multiply (region stride
                                # QC_W + KT_W).
                                j0 = tris[0][1]
                                nb = QC_W // KT_W  # blocks per half
                                op = pt[
                                    :, j0 * KT_W : j0 * KT_W + QC_W + 2 * KT_W
                                ].rearrange("p (x w) -> p x w", w=KT_W)
                                sel = op[:, 0 : nb + 2 : nb + 1]
                                nc.gpsimd.tensor_mul(
                                    sel, sel,
                                    tri_sb.unsqueeze(1).to_broadcast(
                                        [KT_W, 2, KT_W]
                                    ),
                                )
                            else:
                                for h, j in tris:
                                    off = h * QC_W
                                    nc.gpsimd.tensor_mul(
                                        pt[:, off + j * KT_W : off + (j + 1) * KT_W],
                                        pt[:, off + j * KT_W : off + (j + 1) * KT_W],
                                        tri_sb,
                                    )
                            for h, kt in enumerate(pair):
                                cls = block_class[(qc, kt)]
                                if isinstance(cls, tuple) and cls[0] == "m":
                                    mt_ = pt_pool.tile(
                                        [KT_W, QC_W], bf16,
                                        name=f"m01u{sfx}_{pr}_{qc}_{kt}_{lh}",
                                        tag="m01u", bufs=4,
                                    )
                                    nc.sync.dma_start(out=mt_, in_=t_m01[cls[1]])
                                    nc.gpsimd.tensor_mul(
                                        pt[:, h * QC_W : (h + 1) * QC_W],
                                        pt[:, h * QC_W : (h + 1) * QC_W],
                                        mt_,
                                    )

                    cols = sum(QC_W - l for l in leads)
                    return emit, cols * 0.85 + 250.0

                # q-chunks processed DESCENDING (qc=3 first): the final score
                # steps then belong to the smallest chunk (qc=0), minimizing
                # the un-hideable exp tail after the last stream-2 item.
                QC_ORDER = list(range(NQC - 1, -1, -1))
                for qc in QC_ORDER:
                    kts = kts_of[qc]
                    pairs = [tuple(kts[i : i + 2]) for i in range(0, len(kts), 2)]
                    # tri (diagonal) pairs first: their GPSIMD masks are the
                    # deepest dependency of the PV chain.
                    pairs.sort(
                        key=lambda p: 0
                        if any(
                            isinstance(block_class[(qc, kt)], tuple)
                            and block_class[(qc, kt)][0] == "tri"
                            for kt in p
                        )
                        else 1
                    )
                    for pr in range(2):
                        for pair in pairs:
                            for lh in range(2):
                                score_steps.append(_score_step(qc, pr, pair, lh))
                        sc_done[(qc, pr)] = len(score_steps)

                # ----- stream 2: V blocks, PV+normalize halves, out-proj -----
                def _v_item(t):
                    # kt-pair t: s-tiles handled are 128-cols (2t) and (2t+1)
                    # of EVERY kt? No: V-projection output rows are s; tile t
                    # covers s-tiles 2t, 2t+1 for all kt contractions.
                    def emit():
                        vp = v_psum.tile(
                            [128, 2, G, DK], f32, name=f"ps_v{sfx}_{t}", tag="psv"
                        )
                        for m in range(2):
                            st_idx = 2 * t + m
                            xv_t_ = xv_c[st_idx // 4]
                            loc = st_idx % 4
                            for kt in range(NXK):
                                nc.tensor.matmul(
                                    vp[:, m],
                                    xv_t_[:, kt, loc * 128 : (loc + 1) * 128],
                                    wv_sb[:, kt, :],
                                    start=(kt == 0),
                                    stop=(kt == NXK - 1),
                                )
                        # merged bias add for both s-tiles of the pair
                        nc.vector.tensor_add(
                            v2_sb[t][:, :, :, 0:DK], vp,
                            bv_sb.unsqueeze(1).to_broadcast([128, 2, G, DK]),
                        )

                    return emit, 2050.0, (2 if t < 2 else 0)

                def _pv_item(qc, pr, lh):
                    kts = kts_of[qc]

                    def emit():
                        xa = xa_psum.tile(
                            [128, QC_W], f32, name=f"xa{sfx}_{pr}_{qc}_{lh}", tag="xa"
                        )
                        for i, kt in enumerate(kts):
                            lead = _lead(block_class[(qc, kt)])
                            pt, h = pts[(qc, pr, kt, lh)]
                            nc.tensor.matmul(
                                xa[:, lead:QC_W],
                                v_sb(kt)[:, pr * 2 + lh, :],
                                pt[:, h * QC_W + lead : (h + 1) * QC_W],
                                start=(i == 0),
                                stop=(i == len(kts) - 1),
                            )
                        rec = rec_pool.tile(
                            [DK, 2, QC_W], f32, name=f"rec{sfx}_{pr}_{qc}_{lh}", tag="rec"
                        )
                        nc.vector.tensor_copy(rec[:, 1], xa[DK : 2 * DK, :])
                        nc.vector.reciprocal_approx_fast(rec[:, 0], rec[:, 1])
                        rec = rec[:, 0]
                        nc.vector.tensor_mul(
                            xt_sb[pr][
                                lh * DK : (lh + 1) * DK, qc * QC_W : (qc + 1) * QC_W
                            ],
                            xa[0:DK, :],
                            rec,
                        )

                    pe = sum(
                        (QC_W - _lead(block_class[(qc, kt)])) * 0.5 for kt in kts
                    )
                    return emit, pe, sc_done[(qc, pr)]

                ot_stage = {}

                def _o_item(qc, msi):
                    def emit():
                        ms = qc * 4 + msi
                        if msi == 0:
                            ot_stage[qc] = ot_pool.tile(
                                [128, 4, 2, QC_W], bf16, name=f"ot{sfx}_{qc}", tag="ot"
                            )
                        ot = ot_stage[qc]
                        for nc2 in range(2):
                            o = o_psum.tile(
                                [128, QC_W], f32, name=f"o{sfx}_{ms}_{nc2}", tag="o"
                            )
                            for pr in range(2):
                                nc.tensor.matmul(
                                    o,
                                    xt_sb[pr][:, ms * 128 : (ms + 1) * 128],
                                    wo_sb[:, pr, nc2 * QC_W : (nc2 + 1) * QC_W],
                                    start=(pr == 0),
                                    stop=(pr == 1),
                                )
                            nc.vector.tensor_copy(ot[:, msi, nc2], o)
                        if msi == 3:
                            # store on the ACT HWDGE ring: keeps the SP ring
                            # free for the next iteration's input stream.
                            nc.scalar.dma_start(
                                out=t_out[
                                    qc * QC_W : (qc + 1) * QC_W, :
                                ].rearrange("(m p) d -> p m d", p=128),
                                in_=ot.rearrange("p m n w -> p m (n w)"),
                            )

                    return emit, 1000.0, sc_done[(qc, 1)]

                _n = lambda: (lambda: None, 0.0, 0)
                _v_item_ = _v_item if 'v' in parts else lambda t: _n()
                _pv_item_ = _pv_item if 'pv' in parts else lambda qc, pr, lh: _n()
                _o_item_ = _o_item if 'out' in parts else lambda qc, m: _n()
                _v_item, _pv_item, _o_item = _v_item_, _pv_item_, _o_item_
                # Stream 2 follows the descending qc order: pv(3) needs all 8
                # V tiles, so V projection fills the PE while the qc=3 scores
                # stream through exp; out-projections trail their pv's.
                stream2a = [_v_item(t) for t in range(NVP)]
                stream2a += [_pv_item(3, pr, lh) for pr in range(2) for lh in range(2)]
                stream2a += [_pv_item(2, pr, lh) for pr in range(2) for lh in range(2)]
                stream2b = [_o_item(3, m) for m in range(4)]
                stream2b += [_pv_item(1, pr, lh) for pr in range(2) for lh in range(2)]
                stream2b += [_o_item(2, m) for m in range(4)]
                stream2b += [_pv_item(0, pr, lh) for pr in range(2) for lh in range(2)]
                stream2b += [_o_item(1, m) for m in range(4)]
                stream2b += [_o_item(0, m) for m in range(4)]

                state = {"si": 0, "t_exp": 0.0, "t2": 0.0}

                def weave(stream2):
                    for emit2, pe2, min_s in stream2:
                        while state["si"] < len(score_steps) and (
                            state["si"] < min_s
                            or state["t2"] + pe2 > state["t_exp"]
                        ):
                            emit, e = score_steps[state["si"]]
                            emit()
                            state["t_exp"] += e
                            state["si"] += 1
                        emit2()
                        state["t2"] += pe2
                    return

                with tc.tile_pool(name=f"st{sfx}", bufs=2, space="PSUM") as st_psum:
                    with tc.tile_pool(
                        name=f"v_ps{sfx}", bufs=2, space="PSUM"
                    ) as v_psum:
                        weave(stream2a)
                    with tc.tile_pool(
                        name=f"o_ps{sfx}", bufs=2, space="PSUM"
                    ) as o_psum:
                        weave(stream2b)
                        while state["si"] < len(score_steps):
                            emit, e = score_steps[state["si"]]
                            emit()
                            state["si"] += 1

            ctx2.close()
        loop_ctx.close()

    nc.compile()
    return nc


def _classify(mask):
    """Block classification shared by all cores + per-core unique mask tiles.

    Returns (block_class, n_uniq, per_batch_m01) where block_class maps
    (qc, kt) -> 'f' | ('tri', j) | ('m', uniq-index); fully-masked-everywhere
    blocks are absent. per_batch_m01[b] is an [n_uniq, 128, 512] bf16 array.
    """
    mask = np.asarray(mask)
    blk = mask.reshape(B, NQC, QC_W, NKT, KT_W)
    nz = (blk != 0).sum(axis=(2, 4))  # [B, NQC, NKT]
    full = nz == QC_W * KT_W
    empty = nz == 0

    block_class = {}
    uniq = {}
    per_batch = [[] for _ in range(B)]
    qs = np.arange(QC_W)
    ks = np.arange(KT_W)
    for qc in range(NQC):
        for kt in range(NKT):
            if empty[:, qc, kt].all():
                continue
            if full[:, qc, kt].all():
                block_class[(qc, kt)] = "f"
                continue
            blocks = [
                (mask[b, qc * QC_W : (qc + 1) * QC_W, kt * KT_W : (kt + 1) * KT_W] != 0)
                for b in range(B)
            ]
            # causal-shaped block? pattern q >= k with aligned j offset
            j, rem = divmod(kt * KT_W - qc * QC_W, KT_W)
            if rem == 0 and 0 <= j and (j + 1) * KT_W <= QC_W:
                pat = (qc * QC_W + qs)[:, None] >= (kt * KT_W + ks)[None, :]
                if all((blk == pat).all() for blk in blocks):
                    block_class[(qc, kt)] = ("tri", j)
                    continue
            subs = [np.ascontiguousarray(blk.T).astype(BF16) for blk in blocks]
            key = b"".join(s.tobytes() for s in subs)
            if key not in uniq:
                uniq[key] = len(uniq)
                for b in range(B):
                    per_batch[b].append(subs[b])
            block_class[(qc, kt)] = ("m", uniq[key])
    n_uniq = len(uniq)
    m01 = [
        np.stack(per_batch[b]) if n_uniq else None
        for b in range(B)
    ]
    return block_class, n_uniq, m01


def _prep_inputs(query, key, value, mask, Wq, bq, Wk, bk, Wv, bv, Wo, bo):
    """Returns (in_maps, block_class, n_uniq)."""
    block_class, n_uniq, m01 = _classify(mask)
    scale = 1.0 / np.sqrt(np.float32(DK))

    xq = [np.ascontiguousarray(np.asarray(query[b]).T).astype(BF16) for b in range(B)]
    xk = [np.ascontiguousarray(np.asarray(key[b]).T).astype(BF16) for b in range(B)]
    xv = [np.ascontiguousarray(np.asarray(value[b]).T).astype(BF16) for b in range(B)]

    Wq, Wk, Wv, Wo = (np.asarray(a, np.float32) for a in (Wq, Wk, Wv, Wo))
    bq, bk, bv = (np.asarray(a, np.float32) for a in (bq, bk, bv))

    in_maps = []
    for c in range(NCORES):
        b, g = divmod(c, G)
        sl = slice(g * E, (g + 1) * E)
        im = {
            "xq": xq[b],
            "xk": xk[b],
            "xv": xv[b],
            "wq": np.ascontiguousarray(Wq[sl].T * scale).astype(BF16),
            "wk": np.ascontiguousarray(Wk[sl].T).astype(BF16),
            "wv": np.ascontiguousarray(Wv[sl].T).astype(BF16),
            "wo": np.ascontiguousarray(Wo[:, sl].T).astype(BF16),
            "bq": np.ascontiguousarray((bq[sl] * scale).reshape(2, 128).T),
            "bk": np.ascontiguousarray(bk[sl].reshape(2, 128).T),
            "bv": np.ascontiguousarray(bv[sl][None, :]),
        }
        if n_uniq:
            im["m01"] = m01[b]
        if any(isinstance(c2, tuple) and c2[0] == "tri" for c2 in block_class.values()):
            p = np.arange(KT_W)
            im["tri"] = np.ascontiguousarray(
                (p[None, :] >= p[:, None])
            ).astype(BF16)
        in_maps.append(im)
    return in_maps, block_class, n_uniq


_NC_CACHE = {}


def _get_nc(block_class, n_uniq, repeat=1):
    key = (tuple(sorted(block_class.items())), n_uniq, repeat)
    if key not in _NC_CACHE:
        _NC_CACHE[key] = _build_nc(block_class, n_uniq, repeat=repeat)
    return _NC_CACHE[key]


def kernel(query, key, value, mask, Wq, bq, Wk, bk, Wv, bv, Wo, bo):
    in_maps, block_class, n_uniq = _prep_inputs(
        query, key, value, mask, Wq, bq, Wk, bk, Wv, bv, Wo, bo
    )
    nc = _get_nc(block_class, n_uniq)
    res = run_bass_kernel_spmd(nc, in_maps, core_ids=list(range(NCORES)))
    bo = np.asarray(bo, np.float32)
    out = np.empty((B, S, D), np.float32)
    for b in range(B):
        acc = res.results[b * G]["out"].astype(np.float32)
        for g in range(1, G):
            acc = acc + res.results[b * G + g]["out"]
        out[b] = acc + bo[None, :]
    return out


# revision 35
# speedup vs baseline: 1.0602x; 1.0231x over previous
"""Trainium2 Bass kernel for nn_MultiHeadedAttention (B=2, S=2048, D=1024, H=16).

Sharding (8 NeuronCores): tensor-parallel over heads x data-parallel over batch.
Core c handles batch b = c // 4 and head group g = c % 4 (4 heads = 256 dims).

Per-core pipeline (all matmuls bf16, fp32 PSUM accumulation):
  - Inputs streamed in fine chunks: xq (4) + xk (2) on the SP HWDGE ring,
    xv (4 s-chunks) + output stores on the ACT HWDGE ring, so the two rings
    drain in parallel and each consumer starts on its first chunk.
  - Q^T/K^T projections in transposed layout [e, s], contraction(kt)-OUTER so
    matmuls start as soon as the first x-tile DMA lands. Q and K share one
    8-tile PSUM pool (per-tag handoff: K's slot-i matmuls wait only on Q's
    slot-i eviction). Per-partition bias via DVE/ScalarE on eviction;
    1/sqrt(dk) folded into Wq host-side.
  - Scores computed transposed, S^T = K @ Q^T, per (512-q-chunk, 128-k-tile)
    with causally dead leading columns trimmed off the matmul and PV reads.
    The two head-halves (lh) of each step land on PE row-groups 0/64 via
    implicit tile_position, enabling hardware row-tiling concurrency.
    exp on ScalarE straight out of PSUM (scores bounded, no max-subtraction),
    merged into one [128, 1024] instruction per step when the wasted lead
    region is small. Diagonal (tri) pairs are emitted first within each
    (qc, pr) so their GPSIMD masks clear the PV dependency chain early.
  - Mask applied as data on GPSIMD (SBUF-only engine): adjacent-j tri pairs
    masked with a single 2-region strided multiply.
  - V projection [s, e] augmented with 64 REPLICATED ones columns per head:
    the PV matmul then emits the denominator replicated across partitions
    64-127, so normalization is a plain [64,512] reciprocal + multiply on DVE.
    V tiles are allocated per kt-PAIR so the two bias adds merge into one.
  - PV accumulates X^T per (q-chunk, head) with V_aug stationary.
  - Output projection per q-chunk interleaved behind attention; evictions on
    DVE; output DMA issued from the ACT ring; host reduces the 4 head-group
    partials per batch + bo.
"""

import os
from contextlib import ExitStack

import numpy as np
import ml_dtypes

import concourse.bass as bass  # noqa: F401
import concourse.bacc as bacc
import concourse.tile as tile
import concourse.mybir as mybir
from concourse.bass_utils import run_bass_kernel_spmd

dt = mybir.dt
AFT = mybir.ActivationFunctionType
BF16 = ml_dtypes.bfloat16
F8E4 = ml_dtypes.float8_e4m3

# fp8(e4m3) DoubleRow Q/K projections: 2x PE throughput and half the
# xq/xk DMA bytes. W is pre-scaled by 32 (so fp8 values sit in the normal
# range) and the combined 32*32*sqrt(dk) factor is divided out inside the
# exp's scale. Verified numerically: maxrel ~1.5e-2 vs the 2e-2 gate.
FP8_QK = os.environ.get("FP8_QK", "0") == "1"
W_SCALE = 32.0

B, S, D, H = 2, 2048, 1024, 16
DK = D // H                  # 64
NCORES = 8
G = 4                        # heads per core
E = G * DK                   # 256 projected dims per core
QC_W = 512                   # q chunk width
KT_W = 128                   # k tile width
NQC = S // QC_W              # 4
NKT = S // KT_W              # 16
NMS = S // 128               # 16 s-tiles
NXK = D // 128               # 8 contraction tiles for projections
NVP = NMS // 2               # 8 v s-tile-pair tiles


# combined dequant for the fp8 path: W_SCALE^2 from the scaled Q/K weights
# and sqrt(dk) (folded into Wq host-side in the bf16 path instead).
EXP_SCALE = (
    float(1.0 / (W_SCALE * W_SCALE * np.sqrt(np.float32(DK)))) if FP8_QK else 1.0
)


def _lead(cls):
    return cls[1] * KT_W if isinstance(cls, tuple) and cls[0] == "tri" else 0


def _build_nc(block_class, n_uniq, repeat=1, loop_n=0, parts=('qk','v','sc','exp','mask','pv','out')):
    """block_class: dict[(qc,kt)] -> 'f' | ('tri', j) | ('m', idx).
    Fully-masked blocks are absent. Same program for all cores (SPMD)."""
    nc = bacc.Bacc("TRN2", target_bir_lowering=False, debug=False, num_devices=NCORES)

    f32, bf16, f8e4 = dt.float32, dt.bfloat16, dt.float8e4
    qk_dt = f8e4 if FP8_QK else bf16
    if FP8_QK:
        # paired layout for DoubleRow: [p, k2, j, s] with d = k2*256+j*128+p
        t_xq = nc.dram_tensor("xq", [128, NXK // 2, 2, S], f8e4, kind="ExternalInput").ap()
        t_xk = nc.dram_tensor("xk", [128, NXK // 2, 2, S], f8e4, kind="ExternalInput").ap()
        t_wq = nc.dram_tensor("wq", [128, NXK // 2, 2, E], f8e4, kind="ExternalInput").ap()
        t_wk = nc.dram_tensor("wk", [128, NXK // 2, 2, E], f8e4, kind="ExternalInput").ap()
    else:
        t_xq = nc.dram_tensor("xq", [D, S], bf16, kind="ExternalInput").ap()
        t_xk = nc.dram_tensor("xk", [D, S], bf16, kind="ExternalInput").ap()
        t_wq = nc.dram_tensor("wq", [D, E], bf16, kind="ExternalInput").ap()
        t_wk = nc.dram_tensor("wk", [D, E], bf16, kind="ExternalInput").ap()
    t_xv = nc.dram_tensor("xv", [D, S], bf16, kind="ExternalInput").ap()
    t_wv = nc.dram_tensor("wv", [D, E], bf16, kind="ExternalInput").ap()
    t_wo = nc.dram_tensor("wo", [E, D], bf16, kind="ExternalInput").ap()
    t_bq = nc.dram_tensor("bq", [128, 2], f32, kind="ExternalInput").ap()
    t_bk = nc.dram_tensor("bk", [128, 2], f32, kind="ExternalInput").ap()
    t_bv = nc.dram_tensor("bv", [1, E], f32, kind="ExternalInput").ap()
    t_m01 = None
    if n_uniq:
        t_m01 = nc.dram_tensor(
            "m01", [n_uniq, KT_W, QC_W], bf16, kind="ExternalInput"
        ).ap()
    has_tri = any(isinstance(c, tuple) and c[0] == "tri" for c in block_class.values())
    t_tri = None
    if has_tri:
        t_tri = nc.dram_tensor("tri", [KT_W, KT_W], bf16, kind="ExternalInput").ap()
    t_out = nc.dram_tensor("out", [S, D], bf16, kind="ExternalOutput").ap()

    with tile.TileContext(nc) as tc, ExitStack() as ctx:
        singles = ctx.enter_context(tc.tile_pool(name="singles", bufs=1))

        # --- resident weights / biases / mask tiles (loaded once) ---
        if FP8_QK:
            wq_sb = singles.tile([128, NXK // 2, 2, E], f8e4, name="wq_sb")
            wk_sb = singles.tile([128, NXK // 2, 2, E], f8e4, name="wk_sb")
        else:
            wq_sb = singles.tile([128, NXK, E], bf16, name="wq_sb")
            wk_sb = singles.tile([128, NXK, E], bf16, name="wk_sb")
        wv_sb = singles.tile([128, NXK, E], bf16, name="wv_sb")
        wo_sb = singles.tile([128, 2, D], bf16, name="wo_sb")
        bq_sb = singles.tile([128, 2], f32, name="bq_sb")
        bk_sb = singles.tile([128, 2], f32, name="bk_sb")
        bv_sb = singles.tile([128, G, DK], f32, name="bv_sb")
        # wq/bq on the SP ring ahead of the xq stream; the rest on the ACT
        # ring so the first Q-projection matmul starts as early as possible.
        _wq_in = t_wq if FP8_QK else t_wq.rearrange("(k p) e -> p k e", p=128)
        _wk_in = t_wk if FP8_QK else t_wk.rearrange("(k p) e -> p k e", p=128)
        nc.sync.dma_start(out=wq_sb, in_=_wq_in)
        nc.sync.dma_start(out=bq_sb, in_=t_bq)
        nc.scalar.dma_start(out=wk_sb, in_=_wk_in)
        nc.scalar.dma_start(out=bk_sb, in_=t_bk)
        nc.scalar.dma_start(out=wv_sb, in_=t_wv.rearrange("(k p) e -> p k e", p=128))
        nc.scalar.dma_start(
            out=bv_sb, in_=t_bv.rearrange("o (h d) -> o h d", d=DK).to_broadcast([128, G, DK])
        )
        nc.scalar.dma_start(out=wo_sb, in_=t_wo.rearrange("(k p) e -> p k e", p=128))

        tri_sb = None
        if has_tri:
            tri_sb = singles.tile([KT_W, KT_W], bf16, name="tri_sb")
            nc.scalar.dma_start(out=tri_sb, in_=t_tri)

        # --- persistent activations ---
        act_sb = ctx.enter_context(tc.tile_pool(name="act_sb", bufs=1))
        qt_sb = [act_sb.tile([128, S], bf16, name=f"qt_sb{i}") for i in range(2)]
        kt_sb = [act_sb.tile([128, S], bf16, name=f"kt_sb{i}") for i in range(2)]
        # V with 64 replicated ones columns per head, allocated per kt-PAIR:
        # [s-tile 128, pair-slot 2, head, 2*dk]
        v2_sb = [
            act_sb.tile([128, 2, G, 2 * DK], bf16, name=f"v2_sb{i}")
            for i in range(NVP)
        ]
        xt_sb = [act_sb.tile([128, S], bf16, name=f"xt_sb{i}") for i in range(2)]
        for vp_ in range(NVP):
            nc.vector.memset(v2_sb[vp_][:, :, :, DK : 2 * DK], 1.0)

        def v_sb(kt):
            return v2_sb[kt // 2][:, kt % 2]

        import contextlib

        loop_cm = tc.For_i(0, loop_n, 1) if loop_n else contextlib.nullcontext()
        loop_ctx = ExitStack()
        loop_ctx.enter_context(loop_cm)
        for rep in range(repeat):
            sfx = f"r{rep}"
            ctx2 = ExitStack()

            # ---------- input prefetch ----------
            # xq/xk stream on the SP HWDGE ring in fine chunks (projection
            # matmuls start on chunk 0); xv takes the ACT ring in s-chunks so
            # the first V-projection block only waits on its own chunk.
            xv_pool = ctx2.enter_context(tc.tile_pool(name=f"xv{sfx}", bufs=1))
            xv_c = [
                xv_pool.tile([128, NXK, QC_W], bf16, name=f"x_v{sfx}_{c}", tag=f"xv{c}")
                for c in range(4)
            ]
            xk_pool = ctx2.enter_context(
                tc.tile_pool(name=f"xkp{sfx}", bufs=2)
            )
            xqk_pool = ExitStack()
            xqk = xqk_pool.enter_context(tc.tile_pool(name=f"xqk{sfx}", bufs=1))
            xq_c = [
                xqk.tile([128, 2, S], qk_dt, name=f"x_q{sfx}_{c}", tag=f"xq{c}")
                for c in range(4)
            ]
            # xk streamed as 4 q-column slices [128, kt, 512] into a 2-deep
            # rotating pool: each slice feeds the two K-projection items of
            # its 512-column chunk inside the attention weave.
            xk_c = [
                xk_pool.tile(
                    [128, NXK // 2, 2, QC_W] if FP8_QK else [128, NXK, QC_W],
                    qk_dt, name=f"x_k{sfx}_{c}", tag="xkc",
                )
                for c in range(4)
            ]
            if 'qk' in parts:
                if FP8_QK:
                    xq_r = t_xq.rearrange("p c j s -> c p j s")
                    xk_r = t_xk
                else:
                    xq_r = t_xq.rearrange("(c k p) s -> c p k s", c=4, p=128)
                    xk_r = t_xk.rearrange("(k p) s -> p k s", p=128)
                # SP ring order: xq chunks, first two xk slices, xv, last two
                # xk slices (those wait on the rotating slots anyway).
                nc.sync.dma_start(out=xq_c[0], in_=xq_r[0])
                nc.sync.dma_start(out=xq_c[1], in_=xq_r[1])
                nc.sync.dma_start(out=xq_c[2], in_=xq_r[2])
                nc.sync.dma_start(out=xq_c[3], in_=xq_r[3])
                for c in range(2):
                    nc.sync.dma_start(
                        out=xk_c[c],
                        in_=xk_r[..., c * QC_W : (c + 1) * QC_W],
                    )
            if 'v' in parts:
                xv_r = t_xv.rearrange("(k p) s -> p k s", p=128)
                for c in range(4):
                    nc.sync.dma_start(
                        out=xv_c[c], in_=xv_r[:, :, c * QC_W : (c + 1) * QC_W]
                    )
            if 'qk' in parts:
                for c in range(2, 4):
                    nc.sync.dma_start(
                        out=xk_c[c],
                        in_=xk_r[..., c * QC_W : (c + 1) * QC_W],
                    )

            def xq_t(kt):
                return xq_c[kt // 2][:, kt % 2]

            # ---------- Q^T projection, contraction-outer ----------
            # (K is projected inside the attention weave, contraction-inner.)
            NPK = NXK // 2 if FP8_QK else NXK  # contraction steps
            pm = mybir.MatmulPerfMode.DoubleRow if FP8_QK else None
            with tc.tile_pool(name=f"pq{sfx}", bufs=1, space="PSUM") as pp:
                if 'qk' in parts:
                    xts = (
                        [xq_c[k] for k in range(4)] if FP8_QK
                        else [xq_t(k) for k in range(NXK)]
                    )
                    ps = [
                        pp.tile([128, QC_W], f32, name=f"ps_q{sfx}_{i}", tag=f"ps{i}")
                        for i in range(8)
                    ]
                    for kt in range(NPK):
                        for nc4 in range(NQC):
                            for mt in range(2):
                                if FP8_QK:
                                    lhsT = wq_sb[:, kt, :, mt * 128 : (mt + 1) * 128]
                                    rhs = xts[kt][:, :, nc4 * QC_W : (nc4 + 1) * QC_W]
                                else:
                                    lhsT = wq_sb[:, kt, mt * 128 : (mt + 1) * 128]
                                    rhs = xts[kt][:, nc4 * QC_W : (nc4 + 1) * QC_W]
                                nc.tensor.matmul(
                                    ps[nc4 * 2 + mt],
                                    lhsT,
                                    rhs,
                                    start=(kt == 0),
                                    stop=(kt == NPK - 1),
                                    perf_mode=pm,
                                )
                    # evictions alternate DVE/ScalarE (ScalarE idle pre-scores)
                    for nc4 in range(NQC):
                        for mt in range(2):
                            osl = qt_sb[mt][:, nc4 * QC_W : (nc4 + 1) * QC_W]
                            if (nc4 * 2 + mt) % 2 == 0:
                                nc.vector.tensor_scalar_add(
                                    osl, ps[nc4 * 2 + mt], bq_sb[:, mt : mt + 1]
                                )
                            else:
                                nc.scalar.activation(
                                    osl, ps[nc4 * 2 + mt], AFT.Identity,
                                    bias=bq_sb[:, mt : mt + 1],
                                )

            xqk_pool.close()

            # ---------- attention + V projection + output projection ----------
            # Two instruction streams woven by estimated cost so PE and
            # ScalarE stay concurrently busy despite in-order engine queues:
            #   stream 1: score matmuls + exp + mask (exp-bound, 1 step/st-tile)
            #   stream 2: V-projection blocks, PV accumulation, out-projection
            # A stream-2 item is issued once enough exp work is in flight to
            # cover its PE time (plus explicit min-step dependency gates).
            with (
                tc.tile_pool(name=f"pt{sfx}", bufs=38) as pt_pool,
                tc.tile_pool(name=f"xa{sfx}", bufs=2, space="PSUM") as xa_psum,
                tc.tile_pool(name=f"rec{sfx}", bufs=2) as rec_pool,
                tc.tile_pool(name=f"ot{sfx}", bufs=2) as ot_pool,
            ):
                kts_of = {
                    qc: [kt for kt in range(NKT) if (qc, kt) in block_class]
                    for qc in range(NQC)
                }
                pts = {}  # (qc, pr, kt, lh) -> (pt tile, h offset index)

                # ----- stream 1: score matmul + exp + mask steps -----
                score_steps = []  # (emit_fn, exp_cost_ns)
                sc_done = {}  # (qc, pr) -> score step index after which done

                def _score_step(qc, pr, pair, lh):
                    leads = [_lead(block_class[(qc, kt)]) for kt in pair]

                    def emit():
                        st = st_psum.tile(
                            [128, 2 * QC_W], f32,
                            name=f"st{sfx}_{pr}_{qc}_{pair[0]}_{lh}", tag="st",
                        )
                        pt = pt_pool.tile(
                            [128, 2 * QC_W], bf16,
                            name=f"pt{sfx}_{pr}_{qc}_{pair[0]}_{lh}", tag="pt",
                        )
                        for h, kt in enumerate(pair) if 'sc' in parts else ():
                            lead = leads[h]
                            nc.tensor.matmul(
                                st[:, h * QC_W + lead : (h + 1) * QC_W],
                                kt_sb[pr][
                                    lh * DK : (lh + 1) * DK,
                                    kt * KT_W : (kt + 1) * KT_W,
                                ],
                                qt_sb[pr][
                                    lh * DK : (lh + 1) * DK,
                                    qc * QC_W + lead : (qc + 1) * QC_W,
                                ],
                            )
                            pts[(qc, pr, kt, lh)] = (pt, h)
                        if 'exp' in parts:
                            # One merged exp when the unwritten lead region is
                            # small (reading stale PSUM there is safe: those
                            # columns are never consumed downstream).
                            if len(pair) == 2 and sum(leads) <= KT_W:
                                nc.scalar.activation(
                                    pt[:, leads[0] :],
                                    st[:, leads[0] :],
                                    AFT.Exp,
                                    scale=EXP_SCALE,
                                )
                            else:
                                for h in range(len(pair)):
                                    nc.scalar.activation(
                                        pt[:, h * QC_W + leads[h] : (h + 1) * QC_W],
                                        st[:, h * QC_W + leads[h] : (h + 1) * QC_W],
                                        AFT.Exp,
                                        scale=EXP_SCALE,
                                    )
                        if 'mask' in parts:
                            tris = [
                                (h, block_class[(qc, kt)][1])
                                for h, kt in enumerate(pair)
                                if isinstance(block_class[(qc, kt)], tuple)
                                and block_class[(qc, kt)][0] == "tri"
                            ]
                            if (
                                len(tris) == 2
                                and tris[0][1] + 1 == tris[1][1]
                            ):
                                # Adjacent-j pair: both diagonal blocks in one
                                # 2-region strided multiply (region stride
                                # QC_W + KT_W).
                                j0 = tris[0][1]
                                nb = QC_W // KT_W  # blocks per half
                                op = pt[
                                    :, j0 * KT_W : j0 * KT_W + QC_W + 2 * KT_W
                                ].rearrange("p (x w) -> p x w", w=KT_W)
                                sel = op[:, 0 : nb + 2 : nb + 1]
                                nc.gpsimd.tensor_mul(
                                    sel, sel,
                                    tri_sb.unsqueeze(1).to_broadcast(
                                        [KT_W, 2, KT_W]
                                    ),
                                )
                            else:
                                for h, j in tris:
                                    off = h * QC_W
                                    nc.gpsimd.tensor_mul(
                                        pt[:, off + j * KT_W : off + (j + 1) * KT_W],
                                        pt[:, off + j * KT_W : off + (j + 1) * KT_W],
                                        tri_sb,
                                    )
                            for h, kt in enumerate(pair):
                                cls = block_class[(qc, kt)]
                                if isinstance(cls, tuple) and cls[0] == "m":
                                    mt_ = pt_pool.tile(
                                        [KT_W, QC_W], bf16,
                                        name=f"m01u{sfx}_{pr}_{qc}_{kt}_{lh}",
                                        tag="m01u", bufs=4,
                                    )
                                    nc.sync.dma_start(out=mt_, in_=t_m01[cls[1]])
                                    nc.gpsimd.tensor_mul(
                                        pt[:, h * QC_W : (h + 1) * QC_W],
                                        pt[:, h * QC_W : (h + 1) * QC_W],
                                        mt_,
                                    )

                    cols = sum(QC_W - l for l in leads)
                    return emit, cols * 0.85 + 250.0

                QC_ORDER = [0, 1, 2, 3]
                for qc in QC_ORDER:
                    kts = kts_of[qc]
                    pairs = [tuple(kts[i : i + 2]) for i in range(0, len(kts), 2)]
                    # tri (diagonal) pairs first: their GPSIMD masks are the
                    # deepest dependency of the PV chain.
                    pairs.sort(
                        key=lambda p: 0
                        if any(
                            isinstance(block_class[(qc, kt)], tuple)
                            and block_class[(qc, kt)][0] == "tri"
                            for kt in p
                        )
                        else 1
                    )
                    for pr in range(2):
                        for pair in pairs:
                            for lh in range(2):
                                score_steps.append(_score_step(qc, pr, pair, lh))
                        sc_done[(qc, pr)] = len(score_steps)

                # ----- stream 2: K-projection items, V blocks, PV+normalize
                # halves, out-proj -----
                def _k_item(nc4, mt):
                    def emit():
                        kp = k_psum.tile(
                            [128, QC_W], f32, name=f"kp{sfx}_{nc4}_{mt}",
                            tag=f"kp{(nc4 * 2 + mt) % 2}",
                        )
                        for kt in range(NPK):
                            if FP8_QK:
                                lhsT = wk_sb[:, kt, :, mt * 128 : (mt + 1) * 128]
                                rhs = xk_c[nc4][:, kt]
                            else:
                                lhsT = wk_sb[:, kt, mt * 128 : (mt + 1) * 128]
                                rhs = xk_c[nc4][:, kt]
                            nc.tensor.matmul(
                                kp, lhsT, rhs,
                                start=(kt == 0), stop=(kt == NPK - 1),
                                perf_mode=pm,
                            )
                        nc.vector.tensor_scalar_add(
                            kt_sb[mt][:, nc4 * QC_W : (nc4 + 1) * QC_W],
                            kp, bk_sb[:, mt : mt + 1],
                        )

                    pe = NPK * QC_W * (0.2083 if FP8_QK else 0.4167)
                    return emit, pe, 0

                def _v_item(t):
                    # kt-pair t: s-tiles handled are 128-cols (2t) and (2t+1)
                    # of EVERY kt? No: V-projection output rows are s; tile t
                    # covers s-tiles 2t, 2t+1 for all kt contractions.
                    def emit():
                        vp = v_psum.tile(
                            [128, 2, G, DK], f32, name=f"ps_v{sfx}_{t}", tag="psv"
                        )
                        for m in range(2):
                            st_idx = 2 * t + m
                            xv_t_ = xv_c[st_idx // 4]
                            loc = st_idx % 4
                            for kt in range(NXK):
                                nc.tensor.matmul(
                                    vp[:, m],
                                    xv_t_[:, kt, loc * 128 : (loc + 1) * 128],
                                    wv_sb[:, kt, :],
                                    start=(kt == 0),
                                    stop=(kt == NXK - 1),
                                )
                        # merged bias add for both s-tiles of the pair
                        nc.vector.tensor_add(
                            v2_sb[t][:, :, :, 0:DK], vp,
                            bv_sb.unsqueeze(1).to_broadcast([128, 2, G, DK]),
                        )

                    return emit, 2050.0, (2 if t < 2 else 0)

                def _pv_item(qc, pr, lh):
                    kts = kts_of[qc]

                    def emit():
                        xa = xa_psum.tile(
                            [128, QC_W], f32, name=f"xa{sfx}_{pr}_{qc}_{lh}", tag="xa"
                        )
                        for i, kt in enumerate(kts):
                            lead = _lead(block_class[(qc, kt)])
                            pt, h = pts[(qc, pr, kt, lh)]
                            nc.tensor.matmul(
                                xa[:, lead:QC_W],
                                v_sb(kt)[:, pr * 2 + lh, :],
                                pt[:, h * QC_W + lead : (h + 1) * QC_W],
                                start=(i == 0),
                                stop=(i == len(kts) - 1),
                            )
                        rec = rec_pool.tile(
                            [DK, 2, QC_W], f32, name=f"rec{sfx}_{pr}_{qc}_{lh}", tag="rec"
                        )
                        nc.vector.tensor_copy(rec[:, 1], xa[DK : 2 * DK, :])
                        nc.vector.reciprocal_approx_fast(rec[:, 0], rec[:, 1])
                        rec = rec[:, 0]
                        nc.vector.tensor_mul(
                            xt_sb[pr][
                                lh * DK : (lh + 1) * DK, qc * QC_W : (qc + 1) * QC_W
                            ],
                            xa[0:DK, :],
                            rec,
                        )

                    pe = sum(
                        (QC_W - _lead(block_class[(qc, kt)])) * 0.5 for kt in kts
                    )
                    return emit, pe, sc_done[(qc, pr)]

                ot_stage = {}

                def _o_item(qc, msi):
                    def emit():
                        ms = qc * 4 + msi
                        if msi == 0:
                            ot_stage[qc] = ot_pool.tile(
                                [128, 4, 2, QC_W], bf16, name=f"ot{sfx}_{qc}", tag="ot"
                            )
                        ot = ot_stage[qc]
                        for nc2 in range(2):
                            o = o_psum.tile(
                                [128, QC_W], f32, name=f"o{sfx}_{ms}_{nc2}", tag="o"
                            )
                            for pr in range(2):
                                nc.tensor.matmul(
                                    o,
                                    xt_sb[pr][:, ms * 128 : (ms + 1) * 128],
                                    wo_sb[:, pr, nc2 * QC_W : (nc2 + 1) * QC_W],
                                    start=(pr == 0),
                                    stop=(pr == 1),
                                )
                            nc.vector.tensor_copy(ot[:, msi, nc2], o)
                        if msi == 3:
                            # store on the ACT HWDGE ring: keeps the SP ring
                            # free for the next iteration's input stream.
                            nc.scalar.dma_start(
                                out=t_out[
                                    qc * QC_W : (qc + 1) * QC_W, :
                                ].rearrange("(m p) d -> p m d", p=128),
                                in_=ot.rearrange("p m n w -> p m (n w)"),
                            )

                    return emit, 1000.0, sc_done[(qc, 1)]

                _n = lambda: (lambda: None, 0.0, 0)
                _k_item_ = _k_item if 'qk' in parts else lambda nc4, mt: _n()
                _v_item_ = _v_item if 'v' in parts else lambda t: _n()
                _pv_item_ = _pv_item if 'pv' in parts else lambda qc, pr, lh: _n()
                _o_item_ = _o_item if 'out' in parts else lambda qc, m: _n()
                _k_item, _v_item, _pv_item, _o_item = (
                    _k_item_, _v_item_, _pv_item_, _o_item_
                )
                # K items first; the leading two carry pe=0 so the weave
                # emits them before any score step (scores read kt_sb).
                k_raw = [
                    _k_item(nc4, mt) for nc4 in range(NQC) for mt in range(2)
                ]
                stream2k = [
                    (k_raw[0][0], 0.0, 0), (k_raw[1][0], 0.0, 0),
                ] + k_raw[2:]
                # Stream 2 follows QC_ORDER; each pv(qc) trails its scores and
                # each o(qc) is spaced well behind its pv's normalize chain.
                stream2a = [_v_item(0), _v_item(1)]
                stream2a += [_pv_item(0, pr, lh) for pr in range(2) for lh in range(2)]
                stream2a += [_v_item(2), _v_item(3)]
                stream2a += [_pv_item(1, pr, lh) for pr in range(2) for lh in range(2)]
                stream2a += [_v_item(t) for t in range(4, NVP)]
                stream2b = [_o_item(0, m) for m in range(4)]
                stream2b += [_pv_item(2, pr, lh) for pr in range(2) for lh in range(2)]
                stream2b += [_o_item(1, m) for m in range(4)]
                stream2b += [_pv_item(3, pr, lh) for pr in range(2) for lh in range(2)]
                stream2b += [_o_item(2, m) for m in range(4)]
                stream2b += [_o_item(3, m) for m in range(4)]

                state = {"si": 0, "t_exp": 0.0, "t2": 0.0}

                def weave(stream2):
                    for emit2, pe2, min_s in stream2:
                        while state["si"] < len(score_steps) and (
                            state["si"] < min_s
                            or state["t2"] + pe2 > state["t_exp"]
                        ):
                            emit, e = score_steps[state["si"]]
                            emit()
                            state["t_exp"] += e
                            state["si"] += 1
                        emit2()
                        state["t2"] += pe2
                    return

                with tc.tile_pool(name=f"st{sfx}", bufs=2, space="PSUM") as st_psum:
                    with tc.tile_pool(
                        name=f"k_ps{sfx}", bufs=1, space="PSUM"
                    ) as k_psum:
                        weave(stream2k)
                    with tc.tile_pool(
                        name=f"v_ps{sfx}", bufs=2, space="PSUM"
                    ) as v_psum:
                        weave(stream2a)
                    with tc.tile_pool(
                        name=f"o_ps{sfx}", bufs=2, space="PSUM"
                    ) as o_psum:
                        weave(stream2b)
                        while state["si"] < len(score_steps):
                            emit, e = score_steps[state["si"]]
                            emit()
                            state["si"] += 1

            ctx2.close()
        loop_ctx.close()

    nc.compile()
    return nc


def _classify(mask):
    """Block classification shared by all cores + per-core unique mask tiles.

    Returns (block_class, n_uniq, per_batch_m01) where block_class maps
    (qc, kt) -> 'f' | ('tri', j) | ('m', uniq-index); fully-masked-everywhere
    blocks are absent. per_batch_m01[b] is an [n_uniq, 128, 512] bf16 array.
    """
    mask = np.asarray(mask)
    blk = mask.reshape(B, NQC, QC_W, NKT, KT_W)
    nz = (blk != 0).sum(axis=(2, 4))  # [B, NQC, NKT]
    full = nz == QC_W * KT_W
    empty = nz == 0

    block_class = {}
    uniq = {}
    per_batch = [[] for _ in range(B)]
    qs = np.arange(QC_W)
    ks = np.arange(KT_W)
    for qc in range(NQC):
        for kt in range(NKT):
            if empty[:, qc, kt].all():
                continue
            if full[:, qc, kt].all():
                block_class[(qc, kt)] = "f"
                continue
            blocks = [
                (mask[b, qc * QC_W : (qc + 1) * QC_W, kt * KT_W : (kt + 1) * KT_W] != 0)
                for b in range(B)
            ]
            # causal-shaped block? pattern q >= k with aligned j offset
            j, rem = divmod(kt * KT_W - qc * QC_W, KT_W)
            if rem == 0 and 0 <= j and (j + 1) * KT_W <= QC_W:
                pat = (qc * QC_W + qs)[:, None] >= (kt * KT_W + ks)[None, :]
                if all((blk == pat).all() for blk in blocks):
                    block_class[(qc, kt)] = ("tri", j)
                    continue
            subs = [np.ascontiguousarray(blk.T).astype(BF16) for blk in blocks]
            key = b"".join(s.tobytes() for s in subs)
            if key not in uniq:
                uniq[key] = len(uniq)
                for b in range(B):
                    per_batch[b].append(subs[b])
            block_class[(qc, kt)] = ("m", uniq[key])
    n_uniq = len(uniq)
    m01 = [
        np.stack(per_batch[b]) if n_uniq else None
        for b in range(B)
    ]
    return block_class, n_uniq, m01


def _prep_inputs(query, key, value, mask, Wq, bq, Wk, bk, Wv, bv, Wo, bo):
    """Returns (in_maps, block_class, n_uniq)."""
    block_class, n_uniq, m01 = _classify(mask)
    scale = 1.0 / np.sqrt(np.float32(DK))

    xq = [np.ascontiguousarray(np.asarray(query[b]).T).astype(BF16) for b in range(B)]
    xk = [np.ascontiguousarray(np.asarray(key[b]).T).astype(BF16) for b in range(B)]
    xv = [np.ascontiguousarray(np.asarray(value[b]).T).astype(BF16) for b in range(B)]

    Wq, Wk, Wv, Wo = (np.asarray(a, np.float32) for a in (Wq, Wk, Wv, Wo))
    bq, bk, bv = (np.asarray(a, np.float32) for a in (bq, bk, bv))

    def _pair8(aT):
        # [D, N] -> fp8 paired layout [128, D/256, 2, N], d = k2*256 + j*128 + p
        n = aT.shape[1]
        return np.ascontiguousarray(
            np.clip(aT, -224.0, 224.0)
            .reshape(NXK // 2, 2, 128, n)
            .transpose(2, 0, 1, 3)
        ).astype(F8E4)

    if FP8_QK:
        xq8 = [_pair8(np.asarray(query[b], np.float32).T) for b in range(B)]
        xk8 = [_pair8(np.asarray(key[b], np.float32).T) for b in range(B)]

    in_maps = []
    for c in range(NCORES):
        b, g = divmod(c, G)
        sl = slice(g * E, (g + 1) * E)
        im = {
            "xq": xq[b],
            "xk": xk[b],
            "xv": xv[b],
            "wq": np.ascontiguousarray(Wq[sl].T * scale).astype(BF16),
            "wk": np.ascontiguousarray(Wk[sl].T).astype(BF16),
            "wv": np.ascontiguousarray(Wv[sl].T).astype(BF16),
            "wo": np.ascontiguousarray(Wo[:, sl].T).astype(BF16),
            "bq": np.ascontiguousarray((bq[sl] * scale).reshape(2, 128).T),
            "bk": np.ascontiguousarray(bk[sl].reshape(2, 128).T),
            "bv": np.ascontiguousarray(bv[sl][None, :]),
        }
        if FP8_QK:
            im["xq"] = xq8[b]
            im["xk"] = xk8[b]
            im["wq"] = _pair8(Wq[sl].T * W_SCALE)
            im["wk"] = _pair8(Wk[sl].T * W_SCALE)
            im["bq"] = np.ascontiguousarray(
                (bq[sl] * W_SCALE).reshape(2, 128).T
            )
            im["bk"] = np.ascontiguousarray(
                (bk[sl] * W_SCALE).reshape(2, 128).T
            )
        if n_uniq:
            im["m01"] = m01[b]
        if any(isinstance(c2, tuple) and c2[0] == "tri" for c2 in block_class.values()):
            p = np.arange(KT_W)
            im["tri"] = np.ascontiguousarray(
                (p[None, :] >= p[:, None])
            ).astype(BF16)
        in_maps.append(im)
    return in_maps, block_class, n_uniq


_NC_CACHE = {}


def _get_nc(block_class, n_uniq, repeat=1):
    key = (tuple(sorted(block_class.items())), n_uniq, repeat)
    if key not in _NC_CACHE:
        _NC_CACHE[key] = _build_nc(block_class, n_uniq, repeat=repeat)
    return _NC_CACHE[key]


def kernel(query, key, value, mask, Wq, bq, Wk, bk, Wv, bv, Wo, bo):
    in_maps, block_class, n_uniq = _prep_inputs(
        query, key, value, mask, Wq, bq, Wk, bk, Wv, bv, Wo, bo
    )
    nc = _get_nc(block_class, n_uniq)
    res = run_bass_kernel_spmd(nc, in_maps, core_ids=list(range(NCORES)))
    bo = np.asarray(bo, np.float32)
    out = np.empty((B, S, D), np.float32)
    for b in range(B):
        acc = res.results[b * G]["out"].astype(np.float32)
        for g in range(1, G):
            acc = acc + res.results[b * G + g]["out"]
        out[b] = acc + bo[None, :]
    return out
